# revision 52
# baseline (speedup 1.0000x reference)
"""Trainium2 Bass kernel: masked attention with softmax over the query axis (dim 1).

Reference computation (per batch b):
    q = x_q @ Wq.T + bq; k = x_k @ Wk.T + bk; v = x_v @ Wv.T + bv
    score = q @ k.T / sqrt(dk) + (-1e9 where mask==0)
    attn = softmax(score, axis=Sq)   # softmax over the QUERY axis
    y = attn @ v

v4: work is spread across engines per key-chunk (SI_TYPE, measured-balanced):
  'B' si: mask applied ON THE PE -- host ships (1-mask) as fp8e4 bytes and
      each score psum group gets two diag(-192) fp8 matmuls emitted BEFORE
      the f16 qk matmuls (additive mask; exp underflows to exact f16 zero,
      equivalent to the -1e9 mask). den rides on the ACT accumulator
      (accum_out on all four exps) + one tiny DVE reduce. No DVE stt at all.
  'A' si: exp on ACT; mask*attn fused with the den row-sum in one DVE
      scalar_tensor_tensor (u8 mask; stt measured 1x regardless of dtype).
  'D' si: like 'A' but exp computed on the DVE via the exp2 bit trick
      (i16 = round(score*1024*log2e + (15-c)*1024) bitcast to f16;
      rms err ~1.8% on 3/16 of elements -> overall l2 ~7e-3, well under the
      2e-2 gate). D-si are stt-masked so the trick never sees -192 scores.
ScalarE never does identity/bias copies (DVE tensor_scalar handles
projections' psum->sbuf). Mask stays 1 B/elem in DMA either way (25.6 MB
total HBM traffic/core). Engine busy measured: ACT 73us, DVE 67us, PE ~99us
(incl mask matmuls), DMA 92us on a ~130us span.

Sharding: 8 cores = 4 batches x 2 Sk-halves. The softmax axis (Sq) stays whole on
every core so softmax is fully local; each core produces a partial y (sum over its
Sk half) and the host adds the two halves per batch.

Per-core design (everything pre-transposed on the host; all matmuls contract over
the partition axis; zero on-chip transposes):
  - scoreT[s,q] tiles [128s x 512q] = kT.T @ qT with dk=64 on partitions (K=64).
    Score matmuls are ROW-PACKED pairs: rows 0-63 compute even q-chunks, rows
    64-127 odd q-chunks concurrently (qT2/kT2 hold interleaved/duplicated copies
    in both partition halves, produced for free by col-packed projections).
  - exp from [128,1024] PSUM pair-tiles -> fp16 attn tiles (engine per
    SI_TYPE, see above).
  - Phase 1 runs in two q-halves (A: cols 0:sq/2, B: rest) with split
    denominators den = den_a + den_b, so half A starts after only half of x_q
    has streamed in.
  - y^T accumulation col-packed: psum[0:64]=q-chunk j, psum[64:128]=chunk
    j+nq/2; v-projection (bias via a K=1 ones-row matmul) and y are interleaved
    into the half-B loop with lag 2, and lagged consumers are emitted BEFORE
    the current stt so neither the in-order PE queue nor DVE ever stall.
Numerics: fp16 storage, f32 PSUM accumulation (validated resid_var ~7e-7).
"""

import numpy as np

B, SQ, SK, D_MODEL, D_K = 4, 4096, 4096, 1024, 64
N_CORES = 8
SK_SHARD = SK // (N_CORES // B)  # 2048
V_LAG = 2
Y_LAG = 2
MASK_PREFETCH = 6

# Per-si work placement (engine load balancing). Indexed by si % 16.
#  'B': mask on PE (fp8 diag matmul); den: phase-A exps accumulate on the
#       ACT accumulator, phase-B half via one DVE tensor_reduce
#  'A': exp on ACT + mask*den via DVE scalar_tensor_tensor (u8 mask)
SI_TYPE = ['A', 'B', 'D', 'B', 'A', 'B', 'D', 'B',
           'A', 'B', 'D', 'B', 'A', 'B', 'B', 'B']
# fastexp: f16 bit trick  i16 = round(score * 1024*log2(e) + (15 - c)*1024)
FE_SCALE = 1477.319722
FE_BIAS = (15.0 - 0.058) * 1024.0


def emit_kernel(tc, aps, sq, sk, d, dk):
    """Emit the per-core attention kernel into TileContext tc."""
    from contextlib import ExitStack

    from concourse import mybir

    nc = tc.nc
    f16 = mybir.dt.float16
    f32 = mybir.dt.float32
    fp8 = mybir.dt.float8e4
    i16 = mybir.dt.int16
    u8 = mybir.dt.uint8
    AF = mybir.ActivationFunctionType
    ALU = mybir.AluOpType

    def si_type(si):
        return SI_TYPE[si % 16]

    n_d = d // 128            # d_model chunks
    n_si = sk // 128          # key chunks (partition dim of scoreT)
    n_qj = sq // 512          # query chunks of 512
    n_qp = n_qj // 2          # query chunk pairs
    n_sj = sk // 512          # key projection column blocks
    half = sq // 2
    n_vq = max(n_si // 4, 1)  # si per x_v quarter

    x_qP, x_kP, x_vP = aps["x_qP"], aps["x_kP"], aps["x_vP"]
    maskT = aps["maskT"]
    wall = aps["wall"]          # [128, 3, n_d, dk] f16: Wq/Wk/Wv d-chunks
    bias = aps["bias"]          # [128, 2+dk] f32: bq/8 | bk | (unused)
    bv16 = aps["bv16"]          # [1, dk] f16
    out = aps["out"]

    def ts_copy(dst, src, scalar1=1.0, scalar2=None, op0=ALU.mult, op1=None):
        kw = {} if op1 is None else {"op1": op1}
        nc.vector.tensor_scalar(dst, src, scalar1, scalar2, op0=op0, **kw)

    with ExitStack() as ctx:
        const = ctx.enter_context(tc.tile_pool(name="const", bufs=1))
        persist = ctx.enter_context(tc.tile_pool(name="persist", bufs=1))
        stat_p = ctx.enter_context(tc.tile_pool(name="statp", bufs=1))
        mask_p = ctx.enter_context(tc.tile_pool(name="maskp", bufs=8))
        attn_p = ctx.enter_context(tc.tile_pool(name="attnp", bufs=n_si))
        psA = ctx.enter_context(tc.tile_pool(name="psA", bufs=2, space="PSUM"))

        # ---------------- constants ----------------
        # constants ride the Activation HWDGE queue: the sync queue's
        # serial ~0.6us/trigger cost was the cold-start bottleneck (first
        # x-byte landed at ~11us); ACT is idle until the first exp anyway.
        w_sb = const.tile([128, 3, n_d, dk], f16, name="w_sb")
        nc.scalar.dma_start(w_sb[:], wall[:])
        b_sb = const.tile([128, 2 + dk], f32, name="b_sb")
        nc.scalar.dma_start(b_sb[:], bias[:])
        bv_sb = const.tile([1, dk], f16, name="bv_sb")
        nc.scalar.dma_start(bv_sb[:], bv16[:])
        diag_sb = const.tile([128, 128], fp8, name="diag_sb")
        nc.scalar.dma_start(diag_sb[:], aps["dg"].bitcast(fp8))
        ones_sb = const.tile([1, 128], f16, name="ones_sb")
        nc.vector.memset(ones_sb[:], 1.0)
        bq8 = b_sb[:, 0:1]
        bk2 = b_sb[:, 1:2]

        qT2 = persist.tile([128, half], f16, name="qT2")  # top: even, bot: odd
        kT2 = persist.tile([128, sk], f16, name="kT2")    # duplicated halves
        vs_sb = persist.tile([128, n_si, dk], f16, name="vs_sb")
        den_a = stat_p.tile([128, n_si], f32, name="den_a")
        den_b = stat_p.tile([128, n_si], f32, name="den_b")
        den4 = stat_p.tile([128, n_si, 4], f32, name="den4")
        den = stat_p.tile([128, n_si], f32, name="den")
        rec = stat_p.tile([128, n_si], f32, name="rec")

        attn_t = []
        mask_a, mask_b = [], []

        def alloc_mask(lst, si, c0):
            # B-si rows of maskT hold fp8e4 bytes of (1-mask) (PE mask
            # matmul operand); A/D-si rows hold u8 {0,1} mask (DVE stt
            # operand). Same byte count either way.
            dt = fp8 if si_type(si) == 'B' else u8
            mt = mask_p.tile([128, half], dt, name="mask_t")
            nc.sync.dma_start(
                mt[:], maskT[si * 128:(si + 1) * 128, c0:c0 + half].bitcast(dt))
            lst.append(mt)

        with tc.tile_pool(name="xk", bufs=1) as xkp, \
                tc.tile_pool(name="xq", bufs=2) as xqp, \
                tc.tile_pool(name="psP", bufs=2, space="PSUM") as psP:
            # x_k arrives in sj column blocks of 512, x_q in 512-col blocks
            xk_t2, xq_t2 = [], []

            def alloc_xk2(tj):
                # one DMA per 512-col block so kproj(2*tj) can start after
                # only the first 1 MB lands (halves the cold-start latency)
                n = min(2, n_sj - 2 * tj)
                xt = xkp.tile([128, 2, n_d, 512], f16, name="xk_b")
                for b in range(n):
                    nc.sync.dma_start(xt[:, b], x_kP[2 * tj + b])
                xk_t2.append(xt)

            def alloc_xq2(tj):
                # the first two xq tiles ride the Activation HWDGE queue so
                # their triggers + data flow in parallel with the sync
                # queue's xk/mask stream during the cold start; mid-phase
                # tiles stay on sync (ACT triggers there stall the exps)
                eng = nc.scalar if tj < 2 else nc.sync
                n = min(2, n_qj - 2 * tj)
                xt = xqp.tile([128, 2, n_d, 512], f16, name="xq_b")
                for b in range(n):
                    eng.dma_start(xt[:, b], x_qP[2 * tj + b])
                xq_t2.append(xt)

            kproj_ps = {}

            def kproj_part(sj, part):
                # half the d-chunks per call so the PE insertion is small
                if part == 0:
                    kproj_ps[sj] = psP.tile(
                        [128, 512], f32, name="ps_k", tag="psp")
                ps = kproj_ps[sj]
                d0 = part * (n_d // 2)
                for di in range(d0, d0 + n_d // 2):
                    w = w_sb[:, 1, di, :]
                    r = xk_t2[sj // 2][:, sj % 2, di, :]
                    nc.tensor.matmul(ps[0:64, :], w, r, start=(di == 0),
                                     stop=(di == n_d - 1), skip_group_check=True)
                    nc.tensor.matmul(ps[64:128, :], w, r, start=(di == 0),
                                     stop=(di == n_d - 1), skip_group_check=True)
                if part == 1:
                    ts_copy(kT2[:, sj * 512:(sj + 1) * 512], ps[:],
                            bk2, None, op0=ALU.add)

            def kproj(sj):
                kproj_part(sj, 0)
                kproj_part(sj, 1)

            def qproj_pair(p):
                ps = psP.tile([128, 512], f32, name="ps_q", tag="psp")
                for di in range(n_d):
                    w = w_sb[:, 0, di, :]
                    nc.tensor.matmul(
                        ps[0:64, :], w, xq_t2[p][:, 0, di, :],
                        start=(di == 0), stop=(di == n_d - 1),
                        skip_group_check=True)
                    nc.tensor.matmul(
                        ps[64:128, :], w, xq_t2[p][:, 1, di, :],
                        start=(di == 0), stop=(di == n_d - 1),
                        skip_group_check=True)
                ts_copy(qT2[:, p * 512:(p + 1) * 512], ps[:],
                        0.125, bq8, op0=ALU.mult, op1=ALU.add)

            def emit_score(si, p, mlist, p0):
                # [128,1024] psum pair-tile: cols 0:512 = q-chunk 2p (rows
                # 0-63 of qT2), cols 512:1024 = q-chunk 2p+1 (rows 64-127).
                # The two qk matmuls run concurrently (disjoint PE row
                # groups). B-si also get two diag(-192) x maskc fp8 matmuls
                # in the same psum group, emitted FIRST so they can fill PE
                # idle while the previous unit's exp drains the other slot.
                ps = psA.tile([128, 1024], f32, name="ps_s", tag="ps")
                t = si_type(si)
                dst = attn_t[si][:, p * 1024:(p + 1) * 1024]
                if t == 'B':
                    mt = mlist[si]
                    c0 = (p - p0) * 1024
                    nc.tensor.matmul(
                        ps[:, 0:512], diag_sb[:], mt[:, c0:c0 + 512],
                        start=True, stop=False, skip_group_check=True)
                    nc.tensor.matmul(
                        ps[:, 512:1024], diag_sb[:], mt[:, c0 + 512:c0 + 1024],
                        start=True, stop=False, skip_group_check=True)
                qk_start = t != 'B'
                nc.tensor.matmul(
                    ps[:, 0:512], kT2[0:64, si * 128:(si + 1) * 128],
                    qT2[0:64, p * 512:(p + 1) * 512],
                    start=qk_start, stop=True, skip_group_check=True)
                nc.tensor.matmul(
                    ps[:, 512:1024], kT2[64:128, si * 128:(si + 1) * 128],
                    qT2[64:128, p * 512:(p + 1) * 512],
                    start=qk_start, stop=True, skip_group_check=True)
                if t == 'B':
                    nc.scalar.activation(dst, ps[:], AF.Exp,
                                         accum_out=den4[:, si, p:p + 1])
                elif t == 'D':
                    # DVE fastexp (exp2 bit trick, rms err ~1.8%); D-si are
                    # stt-masked so the psum never holds -192 here
                    nc.vector.tensor_scalar(
                        dst.bitcast(i16), ps[:], FE_SCALE, FE_BIAS,
                        op0=ALU.mult, op1=ALU.add)
                else:
                    nc.scalar.activation(dst, ps[:], AF.Exp)

            # ---------------- pre-loop: warm-up + first blocks ----------------
            # dummy K=1 matmuls warm the PE HAM clock (~4us of busy) before
            # the first real projections arrive
            warm = psP.tile([128, 64], f32, name="warm", tag="psp")
            for i in range(40):
                nc.tensor.matmul(warm[:], ones_sb[:], bv_sb[:],
                                 start=(i == 0), stop=(i == 39),
                                 skip_group_check=True)
            # keep the warm-up live past DCE
            nc.vector.tensor_scalar(
                den[:, 0:1], warm[:, 0:1], 0.0, None, op0=ALU.mult)
            alloc_xk2(0)
            kproj(0)
            alloc_xq2(0)
            alloc_mask(mask_a, 0, 0)
            qproj_pair(0)
            for s in range(1, min(MASK_PREFETCH, n_si)):
                alloc_mask(mask_a, s, 0)
            if n_qp > 1:
                alloc_xq2(1)
                qproj_pair(1)
            if n_sj > 2:
                alloc_xk2(1)

            # ---------------- phase 1A: q-half A ----------------
            qproj_done = set()
            for si in range(n_si):
                if si > 0 and si // 2 < n_sj:
                    s, part = si // 2, si % 2
                    if s >= 1:
                        kproj_part(s, part)
                if si == 6 and n_qp > 2:
                    alloc_xq2(2)
                if si == 8 and n_qp > 3:
                    alloc_xq2(3)
                # B-half q projections hidden inside phase A (instead of a
                # serial bubble at the phase boundary)
                if si == n_si - 6 and n_qp > 2:
                    qproj_pair(2)
                    qproj_done.add(2)
                if si == n_si - 4 and n_qp > 3:
                    qproj_pair(3)
                    qproj_done.add(3)
                if si + MASK_PREFETCH < n_si:
                    alloc_mask(mask_a, si + MASK_PREFETCH, 0)
                at = attn_p.tile([128, sq], f16, name="attn_t")
                attn_t.append(at)
                for p in range(n_qp // 2):
                    emit_score(si, p, mask_a, 0)
                if si_type(si) != 'B':
                    # mask multiply + half-A denominator in one DVE stt
                    nc.vector.scalar_tensor_tensor(
                        at[:, 0:half], at[:, 0:half], 1.0, mask_a[si][:],
                        op0=ALU.bypass, op1=ALU.mult,
                        accum_out=den_a[:, si:si + 1])

            for pp in range(2, n_qp):
                if pp not in qproj_done:
                    qproj_pair(pp)

            emit_score_b = emit_score

        # ---------------- between phases: x_v + masks B ----------------
        xvp = ctx.enter_context(tc.tile_pool(name="xv", bufs=2))
        xv_q = []

        def alloc_xv(qi):
            xt = xvp.tile([128, n_d, n_vq * 128], f16, name="xv_t")
            nc.sync.dma_start(xt[:], x_vP[qi])
            xv_q.append(xt)

        def emit_v(si):
            ps = psA.tile([128, dk], f32, name="ps_v", tag="ps")
            xt = xv_q[si // n_vq]
            c0 = (si % n_vq) * 128
            for di in range(n_d):
                nc.tensor.matmul(
                    ps[:], xt[:, di, c0:c0 + 128], w_sb[:, 2, di, :],
                    start=(di == 0), stop=False)
            nc.tensor.matmul(ps[:], ones_sb[:], bv_sb[:], start=False, stop=True)
            ts_copy(vs_sb[:, si, :], ps[:], rec[:, si:si + 1], None,
                    op0=ALU.mult)

        psY = ctx.enter_context(tc.tile_pool(name="psY", bufs=1, space="PSUM"))
        yps = [psY.tile([128, 512], f32, name=f"yps{j}", tag=f"yps{j}")
               for j in range(n_qp)]

        def emit_y(si):
            for j in range(n_qp):
                nc.tensor.matmul(
                    yps[j][0:64, :], vs_sb[:, si, :],
                    attn_t[si][:, j * 512:(j + 1) * 512],
                    start=(si == 0), stop=(si == n_si - 1),
                    skip_group_check=True)
                nc.tensor.matmul(
                    yps[j][64:128, :], vs_sb[:, si, :],
                    attn_t[si][:, (j + n_qp) * 512:(j + n_qp + 1) * 512],
                    start=(si == 0), stop=(si == n_si - 1),
                    skip_group_check=True)

        for s in range(min(MASK_PREFETCH, n_si)):
            alloc_mask(mask_b, s, half)
        alloc_xv(0)
        if n_si > n_vq:
            alloc_xv(1)

        # ---------------- phase 1B: q-half B + interleaved v/y ----------------
        for si in range(n_si):
            if si + MASK_PREFETCH < n_si:
                alloc_mask(mask_b, si + MASK_PREFETCH, half)
            if si >= V_LAG:
                vsi = si - V_LAG
                if vsi % n_vq == 0 and (vsi // n_vq + 2) * n_vq < n_si:
                    alloc_xv(vsi // n_vq + 2)
                emit_v(vsi)
            if si >= Y_LAG:
                emit_y(si - Y_LAG)
            for p in range(n_qp // 2, n_qp):
                emit_score_b(si, p, mask_b, n_qp // 2)
            at = attn_t[si]
            if si_type(si) == 'B':
                nc.vector.tensor_reduce(
                    den[:, si:si + 1], den4[:, si, :],
                    axis=mybir.AxisListType.X, op=ALU.add)
            else:
                nc.vector.scalar_tensor_tensor(
                    at[:, half:sq], at[:, half:sq], 1.0, mask_b[si][:],
                    op0=ALU.bypass, op1=ALU.mult,
                    accum_out=den_b[:, si:si + 1])
                nc.vector.tensor_add(
                    den[:, si:si + 1], den_a[:, si:si + 1],
                    den_b[:, si:si + 1])
            nc.vector.reciprocal(rec[:, si:si + 1], den[:, si:si + 1])
        for si in range(max(n_si - V_LAG, 0), n_si):
            emit_v(si)
        for si in range(max(n_si - Y_LAG, 0), n_si):
            emit_y(si)

        # ---------------- output ----------------
        y_p = ctx.enter_context(tc.tile_pool(name="yp", bufs=1))
        y_sb = y_p.tile([128, half], f16, name="y_sb")
        for j in range(n_qp):
            dst = y_sb[:, j * 512:(j + 1) * 512]
            if j % 2 == 0:
                nc.scalar.activation(dst, yps[j][:], AF.Copy)
            else:
                ts_copy(dst, yps[j][:], 1.0, None, op0=ALU.mult)
        nc.sync.dma_start(out[:], y_sb[:])


def build_nc(sq=SQ, sk=SK_SHARD, d=D_MODEL, dk=D_K):
    """Build + compile the per-core Bacc module."""
    import concourse.tile as tile
    from concourse import bacc, mybir

    f16 = mybir.dt.float16
    f32 = mybir.dt.float32
    u8 = mybir.dt.uint8
    n_d = d // 128

    nc = bacc.Bacc("TRN2", target_bir_lowering=False, debug=False)
    n_vq = max((sk // 128) // 4, 1)
    aps = {
        "x_qP": nc.dram_tensor("x_qP", [sq // 512, 128, n_d, 512], f16,
                               kind="ExternalInput").ap(),
        "x_kP": nc.dram_tensor("x_kP", [sk // 512, 128, n_d, 512], f16,
                               kind="ExternalInput").ap(),
        "x_vP": nc.dram_tensor("x_vP", [4, 128, n_d, n_vq * 128], f16,
                               kind="ExternalInput").ap(),
        "maskT": nc.dram_tensor("maskT", [sk, sq], u8, kind="ExternalInput").ap(),
        "dg": nc.dram_tensor("dg", [128, 128], u8, kind="ExternalInput").ap(),
        "wall": nc.dram_tensor("wall", [128, 3, n_d, dk], f16,
                               kind="ExternalInput").ap(),
        "bias": nc.dram_tensor("bias", [128, 2 + dk], f32,
                               kind="ExternalInput").ap(),
        "bv16": nc.dram_tensor("bv16", [1, dk], f16, kind="ExternalInput").ap(),
        "out": nc.dram_tensor("out", [128, sq // 2], f16,
                              kind="ExternalOutput").ap(),
    }
    with tile.TileContext(nc) as tc:
        emit_kernel(tc, aps, sq, sk, d, dk)
    nc.compile()
    return nc


def pack_cols(xT, block):
    """[d, n] -> [n/block, 128, d/128, block] contiguous: per column-block, the
    exact SBUF tile image ([partition, d-chunk, col])."""
    d, n = xT.shape
    return np.ascontiguousarray(
        xT.reshape(d // 128, 128, n // block, block).transpose(2, 1, 0, 3))


def make_in_maps(x_q, x_k, x_v, mask, Wq, bq, Wk, bk, Wv, bv, sk_shard=SK_SHARD):
    """Host-side sharding + layout prep. Returns list of per-core input dicts."""
    import ml_dtypes

    f16 = np.float16
    fp8 = ml_dtypes.float8_e4m3
    d, dk = Wq.shape[1], Wq.shape[0]
    n_d = d // 128
    n_shards = x_k.shape[1] // sk_shard
    dg = (np.eye(128, dtype=np.float32) * -192.0).astype(fp8).view(np.uint8)

    wall = np.empty((128, 3, n_d, dk), f16)
    for i, W in enumerate((Wq, Wk, Wv)):
        WT = W.T.astype(f16)  # [d, dk]
        for di in range(n_d):
            wall[:, i, di, :] = WT[di * 128:(di + 1) * 128, :]
    bias = np.empty((128, 2 + dk), np.float32)
    bias[:, 0] = np.tile(np.asarray(bq, np.float32) / 8.0, 128 // dk)
    bias[:, 1] = np.tile(np.asarray(bk, np.float32), 128 // dk)
    bias[:, 2:] = np.asarray(bv, np.float32)[None, :]
    bv16 = np.asarray(bv, np.float32).astype(f16).reshape(1, dk)

    n_vq = max((sk_shard // 128) // 4, 1)
    xqP = [pack_cols(x_q[b].T.astype(f16), 512) for b in range(x_q.shape[0])]
    # per-si encoding: B-si rows get fp8e4 bytes of (1-mask); A/D-si rows
    # get u8 {0,1} mask
    m8 = np.asarray(mask, np.int8)
    maskc_f8 = (1 - m8).astype(fp8).view(np.uint8)
    mask_u8 = m8.astype(np.uint8)
    b_rows = np.zeros(sk_shard, bool)
    for si in range(sk_shard // 128):
        if SI_TYPE[si % 16] == 'B':
            b_rows[si * 128:(si + 1) * 128] = True
    in_maps = []
    for b in range(x_q.shape[0]):
        for h in range(n_shards):
            sl = slice(h * sk_shard, (h + 1) * sk_shard)
            in_maps.append({
                "x_qP": xqP[b],
                "x_kP": pack_cols(x_k[b, sl, :].T.astype(f16), 512),
                "x_vP": pack_cols(x_v[b, sl, :].T.astype(f16), n_vq * 128),
                "maskT": np.ascontiguousarray(np.where(
                    b_rows[:, None], maskc_f8[b, :, sl].T,
                    mask_u8[b, :, sl].T)),
                "dg": dg,
                "wall": wall, "bias": bias, "bv16": bv16,
            })
    return in_maps


def unpack_out(o, sq=SQ, dk=D_K):
    """out [128, sq/2] f16 -> yT [dk, sq] f32. Top half: q-chunks 0..nq/2-1,
    bottom half: q-chunks nq/2..nq-1."""
    yT = np.empty((dk, sq), np.float32)
    half = sq // 2
    yT[:, 0:half] = o[0:dk, :].astype(np.float32)
    yT[:, half:sq] = o[64:64 + dk, :].astype(np.float32)
    return yT


_NC_CACHE = {}
# test.py can set extra run_bass_kernel_spmd kwargs here (e.g. trace=True)
RUN_KWARGS = {}


def _get_nc():
    if "nc" not in _NC_CACHE:
        _NC_CACHE["nc"] = build_nc()
    return _NC_CACHE["nc"]


def kernel(**inputs):
    from concourse.bass_utils import run_bass_kernel_spmd

    x_q = np.asarray(inputs["x_q"], np.float32)
    x_k = np.asarray(inputs["x_k"], np.float32)
    x_v = np.asarray(inputs["x_v"], np.float32)
    mask = np.asarray(inputs["mask"])
    Wq, bq = np.asarray(inputs["Wq"], np.float32), np.asarray(inputs["bq"], np.float32)
    Wk, bk = np.asarray(inputs["Wk"], np.float32), np.asarray(inputs["bk"], np.float32)
    Wv, bv = np.asarray(inputs["Wv"], np.float32), np.asarray(inputs["bv"], np.float32)

    nc = _get_nc()
    in_maps = make_in_maps(x_q, x_k, x_v, mask, Wq, bq, Wk, bk, Wv, bv)
    res = run_bass_kernel_spmd(nc, in_maps, list(range(N_CORES)), **RUN_KWARGS)
    _NC_CACHE["last_res"] = res
    n_shards = N_CORES // x_q.shape[0]
    y = np.zeros((x_q.shape[0], SQ, D_K), np.float32)
    for core in range(N_CORES):
        y[core // n_shards] += unpack_out(res.results[core]["out"]).T
    return y



# revision 53
# speedup vs baseline: 1.2531x; 1.2531x over previous
"""Trainium2 Bass kernel: masked attention with softmax over the query axis (dim 1).

Reference computation (per batch b):
    q = x_q @ Wq.T + bq; k = x_k @ Wk.T + bk; v = x_v @ Wv.T + bv
    score = q @ k.T / sqrt(dk) + (-1e9 where mask==0)
    attn = softmax(score, axis=Sq)   # softmax over the QUERY axis
    y = attn @ v

v4: work is spread across engines per key-chunk (SI_TYPE, measured-balanced):
  'B' si: mask applied ON THE PE -- host ships (1-mask) as fp8e4 bytes and
      each score psum group gets two diag(-192) fp8 matmuls emitted BEFORE
      the f16 qk matmuls (additive mask; exp underflows to exact f16 zero,
      equivalent to the -1e9 mask). den rides on the ACT accumulator
      (accum_out on all four exps) + one tiny DVE reduce. No DVE stt at all.
  'A' si: exp on ACT; mask*attn fused with the den row-sum in one DVE
      scalar_tensor_tensor (u8 mask; stt measured 1x regardless of dtype).
  'D' si: like 'A' but exp computed on the DVE via the exp2 bit trick
      (i16 = round(score*1024*log2e + (15-c)*1024) bitcast to f16;
      rms err ~1.8% on 3/16 of elements -> overall l2 ~7e-3, well under the
      2e-2 gate). D-si are stt-masked so the trick never sees -192 scores.
ScalarE never does identity/bias copies (DVE tensor_scalar handles
projections' psum->sbuf). Mask stays 1 B/elem in DMA either way (25.6 MB
total HBM traffic/core). Engine busy measured: ACT 73us, DVE 67us, PE ~99us
(incl mask matmuls), DMA 92us on a ~130us span.

Sharding: 8 cores = 4 batches x 2 Sk-halves. The softmax axis (Sq) stays whole on
every core so softmax is fully local; each core produces a partial y (sum over its
Sk half) and the host adds the two halves per batch.

Per-core design (everything pre-transposed on the host; all matmuls contract over
the partition axis; zero on-chip transposes):
  - scoreT[s,q] tiles [128s x 512q] = kT.T @ qT with dk=64 on partitions (K=64).
    Score matmuls are ROW-PACKED pairs: rows 0-63 compute even q-chunks, rows
    64-127 odd q-chunks concurrently (qT2/kT2 hold interleaved/duplicated copies
    in both partition halves, produced for free by col-packed projections).
  - exp from [128,1024] PSUM pair-tiles -> fp16 attn tiles (engine per
    SI_TYPE, see above).
  - Phase 1 runs in two q-halves (A: cols 0:sq/2, B: rest) with split
    denominators den = den_a + den_b, so half A starts after only half of x_q
    has streamed in.
  - y^T accumulation col-packed: psum[0:64]=q-chunk j, psum[64:128]=chunk
    j+nq/2; v-projection (bias via a K=1 ones-row matmul) and y are interleaved
    into the half-B loop with lag 2, and lagged consumers are emitted BEFORE
    the current stt so neither the in-order PE queue nor DVE ever stall.
Numerics: fp16 storage, f32 PSUM accumulation (validated resid_var ~7e-7).
"""

import numpy as np

B, SQ, SK, D_MODEL, D_K = 4, 4096, 4096, 1024, 64
N_CORES = 8
SK_SHARD = SK // (N_CORES // B)  # 2048
V_LAG = 2
Y_LAG = 3
MASK_PREFETCH = 6

# Per-si work placement (engine load balancing). Indexed by si % 16.
#  'B': mask on PE (fp8 diag matmul); den: phase-A exps accumulate on the
#       ACT accumulator, phase-B half via one DVE tensor_reduce
#  'A': exp on ACT + mask*den via DVE scalar_tensor_tensor (u8 mask)
SI_TYPE = ['A', 'B', 'D', 'B', 'A', 'B', 'D', 'B',
           'A', 'B', 'D', 'B', 'A', 'B', 'B', 'B']
# fastexp: f16 bit trick  i16 = round(score * 1024*log2(e) + (15 - c)*1024)
FE_SCALE = 1477.319722
FE_BIAS = (15.0 - 0.058) * 1024.0


def emit_kernel(tc, aps, sq, sk, d, dk):
    """Emit the per-core attention kernel into TileContext tc."""
    from contextlib import ExitStack

    from concourse import mybir

    nc = tc.nc
    f16 = mybir.dt.float16
    f32 = mybir.dt.float32
    fp8 = mybir.dt.float8e4
    i16 = mybir.dt.int16
    u8 = mybir.dt.uint8
    AF = mybir.ActivationFunctionType
    ALU = mybir.AluOpType

    def si_type(si):
        return SI_TYPE[si % 16]

    n_d = d // 128            # d_model chunks
    n_si = sk // 128          # key chunks (partition dim of scoreT)
    n_qj = sq // 512          # query chunks of 512
    n_qp = n_qj // 2          # query chunk pairs
    n_sj = sk // 512          # key projection column blocks
    half = sq // 2
    n_vq = max(n_si // 4, 1)  # si per x_v quarter

    x_qP, x_kP, x_vP = aps["x_qP"], aps["x_kP"], aps["x_vP"]
    maskT = aps["maskT"]
    wall = aps["wall"]          # [128, 3, n_d, dk] f16: Wq/Wk/Wv d-chunks
    bias = aps["bias"]          # [128, 2+dk] f32: bq/8 | bk | (unused)
    bv16 = aps["bv16"]          # [1, dk] f16
    out = aps["out"]

    def ts_copy(dst, src, scalar1=1.0, scalar2=None, op0=ALU.mult, op1=None):
        kw = {} if op1 is None else {"op1": op1}
        nc.vector.tensor_scalar(dst, src, scalar1, scalar2, op0=op0, **kw)

    with ExitStack() as ctx:
        const = ctx.enter_context(tc.tile_pool(name="const", bufs=1))
        persist = ctx.enter_context(tc.tile_pool(name="persist", bufs=1))
        stat_p = ctx.enter_context(tc.tile_pool(name="statp", bufs=1))
        mask_p = ctx.enter_context(tc.tile_pool(name="maskp", bufs=8))
        attn_p = ctx.enter_context(tc.tile_pool(name="attnp", bufs=n_si))
        psA = ctx.enter_context(tc.tile_pool(name="psA", bufs=2, space="PSUM"))

        # ---------------- constants ----------------
        # constants ride the Activation HWDGE queue: the sync queue's
        # serial ~0.6us/trigger cost was the cold-start bottleneck (first
        # x-byte landed at ~11us); ACT is idle until the first exp anyway.
        w_sb = const.tile([128, 3, n_d, dk], f16, name="w_sb")
        nc.scalar.dma_start(w_sb[:], wall[:])
        b_sb = const.tile([128, 2 + dk], f32, name="b_sb")
        nc.scalar.dma_start(b_sb[:], bias[:])
        bv_sb = const.tile([1, dk], f16, name="bv_sb")
        nc.scalar.dma_start(bv_sb[:], bv16[:])
        diag_sb = const.tile([128, 128], fp8, name="diag_sb")
        nc.scalar.dma_start(diag_sb[:], aps["dg"].bitcast(fp8))
        ones_sb = const.tile([1, 128], f16, name="ones_sb")
        nc.vector.memset(ones_sb[:], 1.0)
        bq8 = b_sb[:, 0:1]
        bk2 = b_sb[:, 1:2]

        qT2 = persist.tile([128, half], f16, name="qT2")  # top: even, bot: odd
        kT2 = persist.tile([128, sk], f16, name="kT2")    # duplicated halves
        vs_sb = persist.tile([128, n_si, dk], f16, name="vs_sb")
        den_a = stat_p.tile([128, n_si], f32, name="den_a")
        den_b = stat_p.tile([128, n_si], f32, name="den_b")
        den4 = stat_p.tile([128, n_si, 4], f32, name="den4")
        den = stat_p.tile([128, n_si], f32, name="den")
        rec = stat_p.tile([128, n_si], f32, name="rec")

        attn_t = []
        mask_a, mask_b = [], []

        def alloc_mask(lst, si, c0):
            # B-si rows of maskT hold fp8e4 bytes of (1-mask) (PE mask
            # matmul operand); A/D-si rows hold u8 {0,1} mask (DVE stt
            # operand). Same byte count either way.
            dt = fp8 if si_type(si) == 'B' else u8
            mt = mask_p.tile([128, half], dt, name="mask_t")
            nc.sync.dma_start(
                mt[:], maskT[si * 128:(si + 1) * 128, c0:c0 + half].bitcast(dt))
            lst.append(mt)

        with tc.tile_pool(name="xk", bufs=1) as xkp, \
                tc.tile_pool(name="xq", bufs=2) as xqp, \
                tc.tile_pool(name="psP", bufs=2, space="PSUM") as psP:
            # x_k arrives in sj column blocks of 512, x_q in 512-col blocks
            xk_t2, xq_t2 = [], []

            def alloc_xk2(tj):
                # one DMA per 512-col block so kproj(2*tj) can start after
                # only the first 1 MB lands (halves the cold-start latency)
                n = min(2, n_sj - 2 * tj)
                xt = xkp.tile([128, 2, n_d, 512], f16, name="xk_b")
                for b in range(n):
                    nc.sync.dma_start(xt[:, b], x_kP[2 * tj + b])
                xk_t2.append(xt)

            def alloc_xq2(tj):
                # the first two xq tiles ride the Activation HWDGE queue so
                # their triggers + data flow in parallel with the sync
                # queue's xk/mask stream during the cold start; mid-phase
                # tiles stay on sync (ACT triggers there stall the exps)
                eng = nc.scalar if tj < 2 else nc.sync
                n = min(2, n_qj - 2 * tj)
                xt = xqp.tile([128, 2, n_d, 512], f16, name="xq_b")
                for b in range(n):
                    eng.dma_start(xt[:, b], x_qP[2 * tj + b])
                xq_t2.append(xt)

            kproj_ps = {}

            def kproj_part(sj, part):
                # half the d-chunks per call so the PE insertion is small
                if part == 0:
                    kproj_ps[sj] = psP.tile(
                        [128, 512], f32, name="ps_k", tag="psp")
                ps = kproj_ps[sj]
                d0 = part * (n_d // 2)
                for di in range(d0, d0 + n_d // 2):
                    w = w_sb[:, 1, di, :]
                    r = xk_t2[sj // 2][:, sj % 2, di, :]
                    nc.tensor.matmul(ps[0:64, :], w, r, start=(di == 0),
                                     stop=(di == n_d - 1), skip_group_check=True)
                    nc.tensor.matmul(ps[64:128, :], w, r, start=(di == 0),
                                     stop=(di == n_d - 1), skip_group_check=True)
                if part == 1:
                    ts_copy(kT2[:, sj * 512:(sj + 1) * 512], ps[:],
                            bk2, None, op0=ALU.add)

            def kproj(sj):
                kproj_part(sj, 0)
                kproj_part(sj, 1)

            def qproj_pair(p):
                ps = psP.tile([128, 512], f32, name="ps_q", tag="psp")
                for di in range(n_d):
                    w = w_sb[:, 0, di, :]
                    nc.tensor.matmul(
                        ps[0:64, :], w, xq_t2[p][:, 0, di, :],
                        start=(di == 0), stop=(di == n_d - 1),
                        skip_group_check=True)
                    nc.tensor.matmul(
                        ps[64:128, :], w, xq_t2[p][:, 1, di, :],
                        start=(di == 0), stop=(di == n_d - 1),
                        skip_group_check=True)
                ts_copy(qT2[:, p * 512:(p + 1) * 512], ps[:],
                        0.125, bq8, op0=ALU.mult, op1=ALU.add)

            def emit_score(si, p, mlist, p0):
                # [128,1024] psum pair-tile: cols 0:512 = q-chunk 2p (rows
                # 0-63 of qT2), cols 512:1024 = q-chunk 2p+1 (rows 64-127).
                # The two qk matmuls run concurrently (disjoint PE row
                # groups). B-si also get two diag(-192) x maskc fp8 matmuls
                # in the same psum group, emitted FIRST so they can fill PE
                # idle while the previous unit's exp drains the other slot.
                ps = psA.tile([128, 1024], f32, name="ps_s", tag="ps")
                t = si_type(si)
                dst = attn_t[si][:, p * 1024:(p + 1) * 1024]
                if t == 'B':
                    mt = mlist[si]
                    c0 = (p - p0) * 1024
                    nc.tensor.matmul(
                        ps[:, 0:512], diag_sb[:], mt[:, c0:c0 + 512],
                        start=True, stop=False, skip_group_check=True)
                    nc.tensor.matmul(
                        ps[:, 512:1024], diag_sb[:], mt[:, c0 + 512:c0 + 1024],
                        start=True, stop=False, skip_group_check=True)
                qk_start = t != 'B'
                nc.tensor.matmul(
                    ps[:, 0:512], kT2[0:64, si * 128:(si + 1) * 128],
                    qT2[0:64, p * 512:(p + 1) * 512],
                    start=qk_start, stop=True, skip_group_check=True)
                nc.tensor.matmul(
                    ps[:, 512:1024], kT2[64:128, si * 128:(si + 1) * 128],
                    qT2[64:128, p * 512:(p + 1) * 512],
                    start=qk_start, stop=True, skip_group_check=True)
                if t == 'B':
                    nc.scalar.activation(dst, ps[:], AF.Exp,
                                         accum_out=den4[:, si, p:p + 1])
                elif t == 'D':
                    # DVE fastexp (exp2 bit trick, rms err ~1.8%); D-si are
                    # stt-masked so the psum never holds -192 here
                    nc.vector.tensor_scalar(
                        dst.bitcast(i16), ps[:], FE_SCALE, FE_BIAS,
                        op0=ALU.mult, op1=ALU.add)
                else:
                    nc.scalar.activation(dst, ps[:], AF.Exp)

            # ---------------- pre-loop: warm-up + first blocks ----------------
            # dummy K=1 matmuls warm the PE HAM clock (~4us of busy) before
            # the first real projections arrive
            warm = psP.tile([128, 64], f32, name="warm", tag="psp")
            for i in range(40):
                nc.tensor.matmul(warm[:], ones_sb[:], bv_sb[:],
                                 start=(i == 0), stop=(i == 39),
                                 skip_group_check=True)
            # keep the warm-up live past DCE
            nc.vector.tensor_scalar(
                den[:, 0:1], warm[:, 0:1], 0.0, None, op0=ALU.mult)
            alloc_xk2(0)
            kproj(0)
            alloc_xq2(0)
            alloc_mask(mask_a, 0, 0)
            qproj_pair(0)
            for s in range(1, min(MASK_PREFETCH, n_si)):
                alloc_mask(mask_a, s, 0)
            if n_qp > 1:
                alloc_xq2(1)
                qproj_pair(1)
            if n_sj > 2:
                alloc_xk2(1)

            # ---------------- phase 1A: q-half A ----------------
            qproj_done = set()
            for si in range(n_si):
                if si > 0 and si // 2 < n_sj:
                    s, part = si // 2, si % 2
                    if s >= 1:
                        kproj_part(s, part)
                if si == 6 and n_qp > 2:
                    alloc_xq2(2)
                if si == 8 and n_qp > 3:
                    alloc_xq2(3)
                # B-half q projections hidden inside phase A (instead of a
                # serial bubble at the phase boundary)
                if si == n_si - 6 and n_qp > 2:
                    qproj_pair(2)
                    qproj_done.add(2)
                if si == n_si - 4 and n_qp > 3:
                    qproj_pair(3)
                    qproj_done.add(3)
                if si + MASK_PREFETCH < n_si:
                    alloc_mask(mask_a, si + MASK_PREFETCH, 0)
                at = attn_p.tile([128, sq], f16, name="attn_t")
                attn_t.append(at)
                for p in range(n_qp // 2):
                    emit_score(si, p, mask_a, 0)
                if si_type(si) != 'B':
                    # mask multiply + half-A denominator in one DVE stt
                    nc.vector.scalar_tensor_tensor(
                        at[:, 0:half], at[:, 0:half], 1.0, mask_a[si][:],
                        op0=ALU.bypass, op1=ALU.mult,
                        accum_out=den_a[:, si:si + 1])

            for pp in range(2, n_qp):
                if pp not in qproj_done:
                    qproj_pair(pp)

            emit_score_b = emit_score

        # ---------------- between phases: x_v + masks B ----------------
        xvp = ctx.enter_context(tc.tile_pool(name="xv", bufs=2))
        xv_q = []

        def alloc_xv(qi):
            xt = xvp.tile([128, n_d, n_vq * 128], f16, name="xv_t")
            nc.sync.dma_start(xt[:], x_vP[qi])
            xv_q.append(xt)

        def emit_v(si):
            ps = psA.tile([128, dk], f32, name="ps_v", tag="ps")
            xt = xv_q[si // n_vq]
            c0 = (si % n_vq) * 128
            for di in range(n_d):
                nc.tensor.matmul(
                    ps[:], xt[:, di, c0:c0 + 128], w_sb[:, 2, di, :],
                    start=(di == 0), stop=False)
            nc.tensor.matmul(ps[:], ones_sb[:], bv_sb[:], start=False, stop=True)
            ts_copy(vs_sb[:, si, :], ps[:], rec[:, si:si + 1], None,
                    op0=ALU.mult)

        psY = ctx.enter_context(tc.tile_pool(name="psY", bufs=1, space="PSUM"))
        yps = [psY.tile([128, 512], f32, name=f"yps{j}", tag=f"yps{j}")
               for j in range(n_qp)]

        def emit_y(si):
            for j in range(n_qp):
                nc.tensor.matmul(
                    yps[j][0:64, :], vs_sb[:, si, :],
                    attn_t[si][:, j * 512:(j + 1) * 512],
                    start=(si == 0), stop=(si == n_si - 1),
                    skip_group_check=True)
                nc.tensor.matmul(
                    yps[j][64:128, :], vs_sb[:, si, :],
                    attn_t[si][:, (j + n_qp) * 512:(j + n_qp + 1) * 512],
                    start=(si == 0), stop=(si == n_si - 1),
                    skip_group_check=True)

        for s in range(min(MASK_PREFETCH, n_si)):
            alloc_mask(mask_b, s, half)
        alloc_xv(0)
        if n_si > n_vq:
            alloc_xv(1)

        # ---------------- phase 1B: q-half B + interleaved v/y ----------------
        for si in range(n_si):
            if si + MASK_PREFETCH < n_si:
                alloc_mask(mask_b, si + MASK_PREFETCH, half)
            if si >= V_LAG:
                vsi = si - V_LAG
                if vsi % n_vq == 0 and (vsi // n_vq + 2) * n_vq < n_si:
                    alloc_xv(vsi // n_vq + 2)
                emit_v(vsi)
            if si >= Y_LAG:
                emit_y(si - Y_LAG)
            for p in range(n_qp // 2, n_qp):
                emit_score_b(si, p, mask_b, n_qp // 2)
            at = attn_t[si]
            if si_type(si) == 'B':
                nc.vector.tensor_reduce(
                    den[:, si:si + 1], den4[:, si, :],
                    axis=mybir.AxisListType.X, op=ALU.add)
            else:
                nc.vector.scalar_tensor_tensor(
                    at[:, half:sq], at[:, half:sq], 1.0, mask_b[si][:],
                    op0=ALU.bypass, op1=ALU.mult,
                    accum_out=den_b[:, si:si + 1])
                nc.vector.tensor_add(
                    den[:, si:si + 1], den_a[:, si:si + 1],
                    den_b[:, si:si + 1])
            nc.vector.reciprocal(rec[:, si:si + 1], den[:, si:si + 1])
        for si in range(max(n_si - V_LAG, 0), n_si):
            emit_v(si)
        for si in range(max(n_si - Y_LAG, 0), n_si):
            emit_y(si)

        # ---------------- output ----------------
        y_p = ctx.enter_context(tc.tile_pool(name="yp", bufs=1))
        y_sb = y_p.tile([128, half], f16, name="y_sb")
        for j in range(n_qp):
            dst = y_sb[:, j * 512:(j + 1) * 512]
            if j % 2 == 0:
                nc.scalar.activation(dst, yps[j][:], AF.Copy)
            else:
                ts_copy(dst, yps[j][:], 1.0, None, op0=ALU.mult)
        nc.sync.dma_start(out[:], y_sb[:])


def build_nc(sq=SQ, sk=SK_SHARD, d=D_MODEL, dk=D_K):
    """Build + compile the per-core Bacc module."""
    import concourse.tile as tile
    from concourse import bacc, mybir

    f16 = mybir.dt.float16
    f32 = mybir.dt.float32
    u8 = mybir.dt.uint8
    n_d = d // 128

    nc = bacc.Bacc("TRN2", target_bir_lowering=False, debug=False)
    n_vq = max((sk // 128) // 4, 1)
    aps = {
        "x_qP": nc.dram_tensor("x_qP", [sq // 512, 128, n_d, 512], f16,
                               kind="ExternalInput").ap(),
        "x_kP": nc.dram_tensor("x_kP", [sk // 512, 128, n_d, 512], f16,
                               kind="ExternalInput").ap(),
        "x_vP": nc.dram_tensor("x_vP", [4, 128, n_d, n_vq * 128], f16,
                               kind="ExternalInput").ap(),
        "maskT": nc.dram_tensor("maskT", [sk, sq], u8, kind="ExternalInput").ap(),
        "dg": nc.dram_tensor("dg", [128, 128], u8, kind="ExternalInput").ap(),
        "wall": nc.dram_tensor("wall", [128, 3, n_d, dk], f16,
                               kind="ExternalInput").ap(),
        "bias": nc.dram_tensor("bias", [128, 2 + dk], f32,
                               kind="ExternalInput").ap(),
        "bv16": nc.dram_tensor("bv16", [1, dk], f16, kind="ExternalInput").ap(),
        "out": nc.dram_tensor("out", [128, sq // 2], f16,
                              kind="ExternalOutput").ap(),
    }
    with tile.TileContext(nc) as tc:
        emit_kernel(tc, aps, sq, sk, d, dk)
    nc.compile()
    return nc


def pack_cols(xT, block):
    """[d, n] -> [n/block, 128, d/128, block] contiguous: per column-block, the
    exact SBUF tile image ([partition, d-chunk, col])."""
    d, n = xT.shape
    return np.ascontiguousarray(
        xT.reshape(d // 128, 128, n // block, block).transpose(2, 1, 0, 3))


def make_in_maps(x_q, x_k, x_v, mask, Wq, bq, Wk, bk, Wv, bv, sk_shard=SK_SHARD):
    """Host-side sharding + layout prep. Returns list of per-core input dicts."""
    import ml_dtypes

    f16 = np.float16
    fp8 = ml_dtypes.float8_e4m3
    d, dk = Wq.shape[1], Wq.shape[0]
    n_d = d // 128
    n_shards = x_k.shape[1] // sk_shard
    dg = (np.eye(128, dtype=np.float32) * -192.0).astype(fp8).view(np.uint8)

    wall = np.empty((128, 3, n_d, dk), f16)
    for i, W in enumerate((Wq, Wk, Wv)):
        WT = W.T.astype(f16)  # [d, dk]
        for di in range(n_d):
            wall[:, i, di, :] = WT[di * 128:(di + 1) * 128, :]
    bias = np.empty((128, 2 + dk), np.float32)
    bias[:, 0] = np.tile(np.asarray(bq, np.float32) / 8.0, 128 // dk)
    bias[:, 1] = np.tile(np.asarray(bk, np.float32), 128 // dk)
    bias[:, 2:] = np.asarray(bv, np.float32)[None, :]
    bv16 = np.asarray(bv, np.float32).astype(f16).reshape(1, dk)

    n_vq = max((sk_shard // 128) // 4, 1)
    xqP = [pack_cols(x_q[b].T.astype(f16), 512) for b in range(x_q.shape[0])]
    # per-si encoding: B-si rows get fp8e4 bytes of (1-mask); A/D-si rows
    # get u8 {0,1} mask
    m8 = np.asarray(mask, np.int8)
    maskc_f8 = (1 - m8).astype(fp8).view(np.uint8)
    mask_u8 = m8.astype(np.uint8)
    b_rows = np.zeros(sk_shard, bool)
    for si in range(sk_shard // 128):
        if SI_TYPE[si % 16] == 'B':
            b_rows[si * 128:(si + 1) * 128] = True
    in_maps = []
    for b in range(x_q.shape[0]):
        for h in range(n_shards):
            sl = slice(h * sk_shard, (h + 1) * sk_shard)
            in_maps.append({
                "x_qP": xqP[b],
                "x_kP": pack_cols(x_k[b, sl, :].T.astype(f16), 512),
                "x_vP": pack_cols(x_v[b, sl, :].T.astype(f16), n_vq * 128),
                "maskT": np.ascontiguousarray(np.where(
                    b_rows[:, None], maskc_f8[b, :, sl].T,
                    mask_u8[b, :, sl].T)),
                "dg": dg,
                "wall": wall, "bias": bias, "bv16": bv16,
            })
    return in_maps


def unpack_out(o, sq=SQ, dk=D_K):
    """out [128, sq/2] f16 -> yT [dk, sq] f32. Top half: q-chunks 0..nq/2-1,
    bottom half: q-chunks nq/2..nq-1."""
    yT = np.empty((dk, sq), np.float32)
    half = sq // 2
    yT[:, 0:half] = o[0:dk, :].astype(np.float32)
    yT[:, half:sq] = o[64:64 + dk, :].astype(np.float32)
    return yT


_NC_CACHE = {}
# test.py can set extra run_bass_kernel_spmd kwargs here (e.g. trace=True)
RUN_KWARGS = {}


def _get_nc():
    if "nc" not in _NC_CACHE:
        _NC_CACHE["nc"] = build_nc()
    return _NC_CACHE["nc"]


def kernel(**inputs):
    from concourse.bass_utils import run_bass_kernel_spmd

    x_q = np.asarray(inputs["x_q"], np.float32)
    x_k = np.asarray(inputs["x_k"], np.float32)
    x_v = np.asarray(inputs["x_v"], np.float32)
    mask = np.asarray(inputs["mask"])
    Wq, bq = np.asarray(inputs["Wq"], np.float32), np.asarray(inputs["bq"], np.float32)
    Wk, bk = np.asarray(inputs["Wk"], np.float32), np.asarray(inputs["bk"], np.float32)
    Wv, bv = np.asarray(inputs["Wv"], np.float32), np.asarray(inputs["bv"], np.float32)

    nc = _get_nc()
    in_maps = make_in_maps(x_q, x_k, x_v, mask, Wq, bq, Wk, bk, Wv, bv)
    res = run_bass_kernel_spmd(nc, in_maps, list(range(N_CORES)), **RUN_KWARGS)
    _NC_CACHE["last_res"] = res
    n_shards = N_CORES // x_q.shape[0]
    y = np.zeros((x_q.shape[0], SQ, D_K), np.float32)
    for core in range(N_CORES):
        y[core // n_shards] += unpack_out(res.results[core]["out"]).T
    return y

